# revision 4
# baseline (speedup 1.0000x reference)
"""Trainium2 Bass kernel for the neural-DAE Euler scan model.

Sharding: pure data parallel — 64 trajectories per NeuronCore x 8 cores.
Layout: activations as [feature_partitions, batch_free] (N=64 per core).

Device computation per core (fp32 throughout):
  - staging tile S [100, 400*64] in SBUF holds per-step inputs (all_init rows
    0:26, ones row 26, raw z/v rows 27:37, masked z/v rows 37:47) and the scan
    state/output rows (x at 64:76, i at 96:100 — 32-aligned so PSUM outputs can
    land on the same partitions via matmul tile_position).
  - hidden activations carry a +1 shift: a~ = elu(h)+1 = min(exp(h),1)+relu(h),
    computed as ACT Exp (bias fused) + DVE tensor_scalar (bias+relu fused) +
    DVE scalar_tensor_tensor (min+add fused); the -1 is folded into the next
    layer's bias (b_eff = b - W.T @ 1, host-precomputed).
  - output layers are fused into the next layer-1 matmuls (M4 = W4@Wa_x,
    Mi = Wa4@Wi_de) so the x/i updates come off the serial critical path.
"""
import sys

for _p in ("/opt/trn_rl_repo", "/opt/trn_rl_repo/concourse"):
    if _p not in sys.path:
        sys.path.insert(0, _p)

import numpy as np

import concourse.bass as bass
import concourse.mybir as mybir
from concourse.tile import TileContext
from concourse.vector_clock import ScopedClock
from concourse.bass_utils import run_bass_kernel_spmd

# ---------------------------------------------------------------------------
# walrus in this container rejects >1 sync wait on one CTRL instruction; split
# the Tile tail-drain waits across nops.
def _patched_drain_and_barrier(self, tick_clock, wait_clock):
    drain_inst = self.nc.sync.drain()
    wait_clock.add_sem_waits(
        drain_inst.ins, ScopedClock({None: tick_clock.global_clock})
    )
    waits = list(drain_inst.ins.sync_info.on_wait)
    if len(waits) > 1:
        drain_inst.ins.sync_info.on_wait = waits[:1]
        for w in waits[1:]:
            nop = self.nc.sync.nop(nofuse=True)
            nop.ins.sync_info = mybir.SyncInfo(on_wait=[w], on_update=[])
    self.nc.all_engine_barrier()
    assert self.sems is not None
    popped = self.nc._tile_sem_poison_stack.pop()
    assert popped is self._sem_poison
    self.nc.clear_and_free_semaphores(list(self.sems.allocated().values()))
    self.nc.all_engine_barrier()


TileContext._drain_and_barrier = _patched_drain_and_barrier

# Same walrus limitation for every instruction: at most one sync wait. Hoist
# extra waits onto same-engine no-ops committed just before the instruction.
_orig_commit = TileContext._commit_instruction


def _patched_commit(self, inst, lazy_reg_writes=True):
    si = getattr(inst, "sync_info", None)
    if si is not None and si.on_wait is not None and len(si.on_wait) > 1:
        waits = list(si.on_wait)
        for w in waits[:-1]:
            nop = mybir.InstNoOp(
                name=self.nc.get_next_instruction_name(),
                engine=inst.engine,
                bass_nofuse=True,
            )
            nop.sync_info = mybir.SyncInfo(on_wait=[w], on_update=[])
            _orig_commit(self, nop, lazy_reg_writes)
        inst.sync_info = mybir.SyncInfo(on_wait=[waits[-1]],
                                        on_update=si.on_update)
    return _orig_commit(self, inst, lazy_reg_writes)


TileContext._commit_instruction = _patched_commit

# ---------------------------------------------------------------------------
X_DIM, Z_DIM, V_DIM, I_DIM, HID = 12, 6, 4, 4, 128
ALL = 26
B, T = 512, 400
NC_CORES = 8
BL = B // NC_CORES          # 64
NB = T * BL                 # 25600
NROW = 100
R_ONE, R_ZV, R_ZVT, R_X, R_I = 26, 27, 37, 64, 96
F32 = mybir.dt.float32
Alu = mybir.AluOpType
Act = mybir.ActivationFunctionType


def _np32(a):
    return np.asarray(a, dtype=np.float32)


def host_prep(inputs):
    """Fused weights + per-core staging arrays (host-side layout prep)."""
    t = _np32(inputs["t"]); z = _np32(inputs["z"]); v = _np32(inputs["v"])
    i = _np32(inputs["i"])
    event_t = _np32(inputs["event_t"])
    z_jump = _np32(inputs["z_jump"]); v_jump = _np32(inputs["v_jump"])
    ip = [(_np32(W), _np32(b)) for W, b in inputs["init_params"]]
    dp = [(_np32(W), _np32(b)) for W, b in inputs["de_params"]]
    ap = [(_np32(W), _np32(b)) for W, b in inputs["ae_params"]]

    one = lambda M: M.sum(axis=0).astype(np.float32)
    W = {}

    W1, b1 = dp[0]
    A, Bm, C = W1[0:ALL], W1[ALL:2 * ALL], W1[2 * ALL:3 * ALL]
    AmB = (A - Bm).astype(np.float32)
    BpC = (Bm + C).astype(np.float32)
    Wx_de, Wzv_de, Wi_de = BpC[0:12], BpC[12:22], BpC[22:26]
    W2d, b2d = dp[1]; W3d, b3d = dp[2]; W4d, b4d = dp[3]
    Wa1, ba1 = ap[0]
    Wa_ai, Wa_x, Wa_z, Wa_v = Wa1[0:26], Wa1[26:38], Wa1[38:44], Wa1[44:48]
    Wa2, ba2 = ap[1]; Wa3, ba3 = ap[2]; Wa4, ba4 = ap[3]
    ba4_eff = (ba4 - one(Wa4)).astype(np.float32)
    b4_eff = (b4d - one(W4d)).astype(np.float32)

    # DE layer-1 "big" lhsT over static rows 0:47
    lhsT_de1 = np.zeros((47, HID), np.float32)
    lhsT_de1[0:26] = AmB
    lhsT_de1[R_ONE] = b1 + Wi_de.T @ ba4_eff
    lhsT_de1[R_ZVT:R_ZVT + 10] = Wzv_de
    W["de1"] = lhsT_de1
    W["wxde"] = Wx_de                                     # [12,128] @ rows 64
    W["mi"] = (Wa4 @ Wi_de).astype(np.float32)
    W["w2d"] = W2d; W["b2d_eff"] = (b2d - one(W2d)).astype(np.float32)
    W["w3d"] = W3d; W["b3d_eff"] = (b3d - one(W3d)).astype(np.float32)
    W["w4d"] = W4d; W["b4_eff"] = b4_eff

    # AE layer-1 "big" lhsT over static rows 0:37
    lhsT_ae1 = np.zeros((37, HID), np.float32)
    lhsT_ae1[0:26] = Wa_ai
    lhsT_ae1[R_ONE] = ba1
    lhsT_ae1[R_ZV:R_ZV + 6] = Wa_z
    lhsT_ae1[R_ZV + 6:R_ZV + 10] = Wa_v
    W["ae1"] = lhsT_ae1
    W["wax"] = Wa_x                                       # [12,128] @ rows 64
    W["m4"] = (W4d @ Wa_x).astype(np.float32)
    W["ca2"] = (Wa_x.T @ b4_eff).astype(np.float32)
    W["wa2"] = Wa2; W["ba2_eff"] = (ba2 - one(Wa2)).astype(np.float32)
    W["wa3"] = Wa3; W["ba3_eff"] = (ba3 - one(Wa3)).astype(np.float32)
    W["wa4"] = Wa4; W["ba4_eff"] = ba4_eff

    Wi1, bi1 = ip[0]; Wi2, bi2 = ip[1]; Wi3, bi3 = ip[2]
    wi1 = np.zeros((15, HID), np.float32)
    wi1[0:14] = Wi1
    wi1[14] = bi1
    W["wi1"] = wi1
    W["wi2"] = Wi2; W["bi2_eff"] = (bi2 - one(Wi2)).astype(np.float32)
    W["wi3"] = Wi3; W["bi3_eff"] = (bi3 - one(Wi3)).astype(np.float32)

    tg = t[0, :, 0]
    dt_row = np.zeros((T,), np.float32)
    dt_row[0:T - 1] = tg[1:] - tg[:-1]

    stages = []
    for c in range(NC_CORES):
        bs = slice(c * BL, (c + 1) * BL)
        S = np.zeros((NROW, T, BL), np.float32)
        zc = z[bs].transpose(2, 1, 0)
        vc = v[bs].transpose(2, 1, 0)
        ic = i[bs].transpose(2, 1, 0)
        S[12:18] = zc[:, 0:1, :]
        S[18:22] = vc[:, 0:1, :]
        S[22:26] = ic[:, 0:1, :]
        S[R_ONE] = 1.0
        S[R_ZV:R_ZV + 6] = zc
        S[R_ZV + 6:R_ZV + 10] = vc
        ev = event_t[bs, 0]
        tk = tg[:-1][:, None]; tk1 = tg[1:][:, None]
        mask = (ev[None, :] > tk) & (ev[None, :] <= tk1)
        zj = z_jump[bs].T[:, None, :]; vj = v_jump[bs].T[:, None, :]
        S[R_ZVT:R_ZVT + 6, :T - 1] = np.where(mask[None], zj, zc[:, :-1])
        S[R_ZVT + 6:R_ZVT + 10, :T - 1] = np.where(mask[None], vj, vc[:, :-1])
        stages.append(np.ascontiguousarray(S.reshape(NROW, NB)))
    return W, stages, dt_row


# --- weight pack column map ------------------------------------------------
_COLS = {}


def _build_colmap():
    cur = 0
    for name, w in [("de1", 128), ("mi", 128), ("w2d", 128), ("w3d", 128),
                    ("w4d", 12), ("ae1", 128), ("wxde", 128), ("wax", 128),
                    ("m4", 128), ("wa2", 128), ("wa3", 128), ("wa4", 4),
                    ("wi1", 128), ("wi2", 128), ("wi3", 12), ("ones", 128),
                    ("ca2row", 128), ("b4row", 12), ("dt", T), ("bc", 7)]:
        _COLS[name] = (cur, w)
        cur += w
    return cur


NW = _build_colmap()


def pack_weights(W, dt_row):
    P = np.zeros((128, NW), np.float32)

    def put(name, arr, r0=0):  # arr [K, width]
        c0, w = _COLS[name]
        arr = np.asarray(arr, np.float32)
        assert arr.shape[1] == w, (name, arr.shape)
        P[r0:r0 + arr.shape[0], c0:c0 + w] = arr

    put("de1", W["de1"]); put("mi", W["mi"]); put("w2d", W["w2d"])
    put("w3d", W["w3d"]); put("w4d", W["w4d"]); put("ae1", W["ae1"])
    put("wxde", W["wxde"], r0=R_X); put("wax", W["wax"], r0=R_X)
    put("m4", W["m4"]); put("wa2", W["wa2"]); put("wa3", W["wa3"])
    put("wa4", W["wa4"]); put("wi1", W["wi1"]); put("wi2", W["wi2"])
    put("wi3", W["wi3"])
    put("ones", np.ones((1, 128), np.float32))
    put("ca2row", W["ca2"][None, :])
    put("b4row", W["b4_eff"][None, :])
    put("dt", dt_row[None, :])
    bc = np.zeros((128, 7), np.float32)
    bc[:, 0] = W["b2d_eff"]; bc[:, 1] = W["b3d_eff"]
    bc[:, 2] = W["ba2_eff"]; bc[:, 3] = W["ba3_eff"]
    bc[:, 4] = W["bi2_eff"]; bc[0:12, 5] = W["bi3_eff"]
    bc[R_I:R_I + 4, 6] = W["ba4_eff"]
    put("bc", bc)
    return P


# --- device kernel ---------------------------------------------------------
_NC_CACHE = []


def build_bass():
    nc = bass.Bass()
    stage_d = nc.dram_tensor("stage", [NROW, NB], F32, kind="ExternalInput")
    wpack_d = nc.dram_tensor("wpack", [128, NW], F32, kind="ExternalInput")
    out_d = nc.dram_tensor("out", [16, NB], F32, kind="ExternalOutput")

    with TileContext(nc) as tc:
        with (
            tc.tile_pool(name="pers", bufs=1) as pers,
            tc.tile_pool(name="work", bufs=2) as work,
            tc.tile_pool(name="psum", bufs=1, space="PSUM") as psum,
        ):
            wt = pers.tile([128, NW], F32, tag="wt")
            S = pers.tile([NROW, NB], F32, tag="S")
            nc.gpsimd.dma_start(wt[:, :], wpack_d[:, :])
            nc.gpsimd.dma_start(S[:, :], stage_d[:, :])

            def ws(name, r0, r1, c0=0, cn=None):
                cc0, w = _COLS[name]
                cn = w if cn is None else cn
                return wt[r0:r1, cc0 + c0:cc0 + c0 + cn]

            bc0, _ = _COLS["bc"]

            def bcol(j, p0=0, p1=128):
                return wt[p0:p1, bc0 + j:bc0 + j + 1]

            # ---- derived per-step tables (outer products with dt row) ----
            dtb128 = pers.tile([128, T], F32, tag="dtb128")
            dtbx = pers.tile([128, T], F32, tag="dtbx")     # rows 64:76 used
            ae1b = pers.tile([128, T], F32, tag="ae1b")
            ps = psum.tile([128, T], F32, tag="d1")
            nc.tensor.matmul(ps[:, :], ws("ones", 0, 1), ws("dt", 0, 1),
                             start=True, stop=True)
            nc.vector.tensor_copy(dtb128[:, :], ps[:, :])
            ps = psum.tile([128, T], F32, tag="d2")
            nc.tensor.matmul(ps[R_X:R_X + 12, :], ws("b4row", 0, 1),
                             ws("dt", 0, 1), start=True, stop=True,
                             tile_position=(0, R_X))
            nc.vector.tensor_copy(dtbx[R_X:R_X + 12, :], ps[R_X:R_X + 12, :])
            ps = psum.tile([128, T], F32, tag="d3")
            nc.tensor.matmul(ps[:, :], ws("ca2row", 0, 1), ws("dt", 0, 1),
                             start=True, stop=True)
            nc.vector.tensor_copy(ae1b[:, :], ps[:, :])

            def blk(r0, r1, k):
                return S[r0:r1, k * BL:(k + 1) * BL]

            def elu(ps_t, bias, tag, p=128):
                E = work.tile([p, BL], F32, tag=f"E{tag}")
                r = work.tile([p, BL], F32, tag=f"r{tag}")
                a = work.tile([p, BL], F32, tag=f"a{tag}")
                if bias is None:
                    nc.scalar.activation(E[:, :], ps_t[:, :], Act.Exp)
                    nc.vector.tensor_scalar(r[:, :], ps_t[:, :], 0.0, None,
                                            Alu.max)
                else:
                    nc.scalar.activation(E[:, :], ps_t[:, :], Act.Exp,
                                         bias=bias)
                    nc.vector.tensor_scalar(r[:, :], ps_t[:, :], bias, 0.0,
                                            Alu.add, Alu.max)
                nc.vector.scalar_tensor_tensor(a[:, :], E[:, :], 1.0, r[:, :],
                                               Alu.min, Alu.add)
                return a

            # ---- preamble: init MLP -> x0 ----
            scr = pers.tile([15, BL], F32, tag="scr")
            nc.gpsimd.dma_start(scr[0:10, :], blk(R_ZV, R_ZV + 10, 0))
            nc.gpsimd.dma_start(scr[10:14, :], blk(22, 26, 0))
            nc.gpsimd.dma_start(scr[14:15, :], blk(R_ONE, R_ONE + 1, 0))
            p1 = psum.tile([128, BL], F32, tag="a1")
            nc.tensor.matmul(p1[:, :], ws("wi1", 0, 15), scr[:, :],
                             start=True, stop=True)
            ai1 = elu(p1, None, "a1")
            p2 = psum.tile([128, BL], F32, tag="a2")
            nc.tensor.matmul(p2[:, :], ws("wi2", 0, 128), ai1[:, :],
                             start=True, stop=True)
            ai2 = elu(p2, bcol(4), "a2")
            p3 = psum.tile([12, BL], F32, tag="x4")
            nc.tensor.matmul(p3[:, :], ws("wi3", 0, 128), ai2[:, :],
                             start=True, stop=True)
            x0t = pers.tile([12, BL], F32, tag="x0t")
            nc.vector.tensor_scalar(x0t[:, :], p3[:, :], bcol(5, 0, 12), None,
                                    Alu.add)
            nc.gpsimd.dma_start(blk(R_X, R_X + 12, 0), x0t[:, :])
            nc.gpsimd.dma_start(S[0:12, 0:BL], x0t[:, :])
            # broadcast x0 across all T blocks (doubling SBUF->SBUF DMAs)
            w = BL
            while w < NB:
                nw = min(w, NB - w)
                nc.gpsimd.dma_start(S[0:12, w:w + nw], S[0:12, 0:nw])
                w += nw

            # ---- preamble: init AE -> i0_hat ----
            pa = psum.tile([128, BL], F32, tag="a1")
            nc.tensor.matmul(pa[:, :], ws("ae1", 0, 37), blk(0, 37, 0),
                             start=True, stop=False)
            nc.tensor.matmul(pa[:, :], ws("wax", R_X, R_X + 12),
                             blk(R_X, R_X + 12, 0), start=False, stop=True)
            aa1 = elu(pa, ae1b[:, T - 1:T], "a1e")
            pa2 = psum.tile([128, BL], F32, tag="a2")
            nc.tensor.matmul(pa2[:, :], ws("wa2", 0, 128), aa1[:, :],
                             start=True, stop=True)
            aa2 = elu(pa2, bcol(2), "a2e")
            pa3 = psum.tile([128, BL], F32, tag="a3")
            nc.tensor.matmul(pa3[:, :], ws("wa3", 0, 128), aa2[:, :],
                             start=True, stop=True)
            aa3 = elu(pa3, bcol(3), "a3e")
            pi4 = psum.tile([128, BL], F32, tag="i4")
            nc.tensor.matmul(pi4[R_I:R_I + 4, :], ws("wa4", 0, 128),
                             aa3[:, :], start=True, stop=True,
                             tile_position=(0, R_I))
            nc.scalar.activation(blk(R_I, R_I + 4, 0), pi4[R_I:R_I + 4, :],
                                 Act.Identity, bias=bcol(6, R_I, R_I + 4))

            # ---- scan ----
            for k in range(T - 1):
                pd1 = psum.tile([128, BL], F32, tag="d1")
                nc.tensor.matmul(pd1[:, :], ws("de1", 0, 47), blk(0, 47, k),
                                 start=True, stop=False)
                nc.tensor.matmul(pd1[:, :], ws("wxde", R_X, R_X + 12),
                                 blk(R_X, R_X + 12, k), start=False,
                                 stop=False)
                nc.tensor.matmul(pd1[:, :], ws("mi", 0, 128), aa3[:, :],
                                 start=False, stop=True)
                ad1 = elu(pd1, None, "d1")
                pd2 = psum.tile([128, BL], F32, tag="d2")
                nc.tensor.matmul(pd2[:, :], ws("w2d", 0, 128), ad1[:, :],
                                 start=True, stop=True)
                ad2 = elu(pd2, bcol(0), "d2")
                pd3 = psum.tile([128, BL], F32, tag="d3")
                nc.tensor.matmul(pd3[:, :], ws("w3d", 0, 128), ad2[:, :],
                                 start=True, stop=True)
                ad3 = elu(pd3, bcol(1), "d3")
                s3 = work.tile([128, BL], F32, tag="s3")
                nc.vector.tensor_scalar(s3[:, :], ad3[:, :],
                                        dtb128[:, k:k + 1], None, Alu.mult)
                px4 = psum.tile([128, BL], F32, tag="x4")
                nc.tensor.matmul(px4[R_X:R_X + 12, :], ws("w4d", 0, 128),
                                 s3[:, :], start=True, stop=True,
                                 tile_position=(0, R_X))
                nc.vector.scalar_tensor_tensor(
                    blk(R_X, R_X + 12, k + 1), px4[R_X:R_X + 12, :],
                    dtbx[R_X:R_X + 12, k:k + 1], blk(R_X, R_X + 12, k),
                    Alu.add, Alu.add)
                pa1 = psum.tile([128, BL], F32, tag="a1")
                nc.tensor.matmul(pa1[:, :], ws("ae1", 0, 37),
                                 blk(0, 37, k + 1), start=True, stop=False)
                nc.tensor.matmul(pa1[:, :], ws("wax", R_X, R_X + 12),
                                 blk(R_X, R_X + 12, k), start=False,
                                 stop=False)
                nc.tensor.matmul(pa1[:, :], ws("m4", 0, 128), s3[:, :],
                                 start=False, stop=True)
                aa1 = elu(pa1, ae1b[:, k:k + 1], "a1e")
                pa2 = psum.tile([128, BL], F32, tag="a2")
                nc.tensor.matmul(pa2[:, :], ws("wa2", 0, 128), aa1[:, :],
                                 start=True, stop=True)
                aa2 = elu(pa2, bcol(2), "a2e")
                pa3 = psum.tile([128, BL], F32, tag="a3")
                nc.tensor.matmul(pa3[:, :], ws("wa3", 0, 128), aa2[:, :],
                                 start=True, stop=True)
                aa3 = elu(pa3, bcol(3), "a3e")
                pi4 = psum.tile([128, BL], F32, tag="i4")
                nc.tensor.matmul(pi4[R_I:R_I + 4, :], ws("wa4", 0, 128),
                                 aa3[:, :], start=True, stop=True,
                                 tile_position=(0, R_I))
                nc.scalar.activation(blk(R_I, R_I + 4, k + 1),
                                     pi4[R_I:R_I + 4, :], Act.Identity,
                                     bias=bcol(6, R_I, R_I + 4))

            nc.gpsimd.dma_start(out_d[0:12, :], S[R_X:R_X + 12, :])
            nc.gpsimd.dma_start(out_d[12:16, :], S[R_I:R_I + 4, :])
    return nc


def kernel(**inputs):
    W, stages, dt_row = host_prep(inputs)
    wpack = pack_weights(W, dt_row)
    if not _NC_CACHE:
        _NC_CACHE.append(build_bass())
    nc = _NC_CACHE[0]
    in_maps = [{"stage": stages[c], "wpack": wpack} for c in range(NC_CORES)]
    res = run_bass_kernel_spmd(nc, in_maps, core_ids=list(range(NC_CORES)))
    xs, is_ = [], []
    for c in range(NC_CORES):
        o = res.results[c]["out"].reshape(16, T, BL)
        xs.append(o[0:12].transpose(2, 1, 0))
        is_.append(o[12:16].transpose(2, 1, 0))
    return (np.ascontiguousarray(np.concatenate(xs, 0)),
            np.ascontiguousarray(np.concatenate(is_, 0)))


# revision 5
# speedup vs baseline: 1.3497x; 1.3497x over previous
"""Trainium2 Bass kernel for the neural-DAE Euler scan model.

Sharding: pure data parallel — 64 trajectories per NeuronCore x 8 cores.
Layout: activations as [feature_partitions, batch_free] (N=64 per core).

Device computation per core:
  - staging tile S [100, 400*64] fp32 holds per-step inputs and the scan
    state/output rows (x at 64:76, i at 96:100 — 32-aligned so PSUM outputs
    land on the same partitions via matmul tile_position); S_bf [47, .] bf16
    mirrors the static rows for the fast layer-1 matmuls.
  - hidden-layer matmuls run in bf16 (weights + activations); the Euler x/i
    state, all biases, and the dx path (w4d/wxde/wax) stay fp32.
  - hidden activations carry a +1 shift: a~ = elu(h)+1 = max(h+1, exp(min(h,0)))
    via DVE tensor_scalar (bias+min fused) -> ACT Exp (SBUF only, so the PSUM
    co-reader serialization Tile imposes on ACT+DVE never triggers) -> DVE
    scalar_tensor_tensor (bias+1+max fused, casts to bf16); the -1 shift is
    folded into the next layer's bias (b_eff = b - W.T @ 1).
  - output layers are fused into the next layer-1 matmuls (M4 = W4@Wa_x,
    Mi = Wa4@Wi_de) so the x/i updates come off the serial critical path.
"""
import sys

for _p in ("/opt/trn_rl_repo", "/opt/trn_rl_repo/concourse"):
    if _p not in sys.path:
        sys.path.insert(0, _p)

import numpy as np
import ml_dtypes

import concourse.bass as bass
import concourse.mybir as mybir
from concourse.tile import TileContext
from concourse.vector_clock import ScopedClock
from concourse.bass_utils import run_bass_kernel_spmd

BF16 = ml_dtypes.bfloat16

# ---------------------------------------------------------------------------
# walrus in this container rejects >1 sync wait on one instruction; split the
# Tile tail-drain waits across nops, and hoist extra per-instruction waits
# onto same-engine no-ops at lowering time.
def _patched_drain_and_barrier(self, tick_clock, wait_clock):
    drain_inst = self.nc.sync.drain()
    wait_clock.add_sem_waits(
        drain_inst.ins, ScopedClock({None: tick_clock.global_clock})
    )
    waits = list(drain_inst.ins.sync_info.on_wait)
    if len(waits) > 1:
        drain_inst.ins.sync_info.on_wait = waits[:1]
        for w in waits[1:]:
            nop = self.nc.sync.nop(nofuse=True)
            nop.ins.sync_info = mybir.SyncInfo(on_wait=[w], on_update=[])
    self.nc.all_engine_barrier()
    assert self.sems is not None
    popped = self.nc._tile_sem_poison_stack.pop()
    assert popped is self._sem_poison
    self.nc.clear_and_free_semaphores(list(self.sems.allocated().values()))
    self.nc.all_engine_barrier()


TileContext._drain_and_barrier = _patched_drain_and_barrier

_orig_commit = TileContext._commit_instruction


def _patched_commit(self, inst, lazy_reg_writes=True):
    si = getattr(inst, "sync_info", None)
    if si is not None and si.on_wait is not None and len(si.on_wait) > 1:
        waits = list(si.on_wait)
        for w in waits[:-1]:
            nop = mybir.InstNoOp(
                name=self.nc.get_next_instruction_name(),
                engine=inst.engine,
                bass_nofuse=True,
            )
            nop.sync_info = mybir.SyncInfo(on_wait=[w], on_update=[])
            _orig_commit(self, nop, lazy_reg_writes)
        inst.sync_info = mybir.SyncInfo(on_wait=[waits[-1]],
                                        on_update=si.on_update)
    return _orig_commit(self, inst, lazy_reg_writes)


TileContext._commit_instruction = _patched_commit

# ---------------------------------------------------------------------------
X_DIM, Z_DIM, V_DIM, I_DIM, HID = 12, 6, 4, 4, 128
ALL = 26
B, T = 512, 400
NC_CORES = 8
BL = B // NC_CORES          # 64
NB = T * BL                 # 25600
NROW = 100
R_ONE, R_ZV, R_ZVT, R_X, R_I = 26, 27, 37, 64, 96
F32 = mybir.dt.float32
BF = mybir.dt.bfloat16
Alu = mybir.AluOpType
Act = mybir.ActivationFunctionType


def _np32(a):
    return np.asarray(a, dtype=np.float32)


def host_prep(inputs):
    """Fused weights + per-core staging arrays (host-side layout prep)."""
    t = _np32(inputs["t"]); z = _np32(inputs["z"]); v = _np32(inputs["v"])
    i = _np32(inputs["i"])
    event_t = _np32(inputs["event_t"])
    z_jump = _np32(inputs["z_jump"]); v_jump = _np32(inputs["v_jump"])
    ip = [(_np32(W), _np32(b)) for W, b in inputs["init_params"]]
    dp = [(_np32(W), _np32(b)) for W, b in inputs["de_params"]]
    ap = [(_np32(W), _np32(b)) for W, b in inputs["ae_params"]]

    one = lambda M: M.sum(axis=0).astype(np.float32)
    W = {}

    W1, b1 = dp[0]
    A, Bm, C = W1[0:ALL], W1[ALL:2 * ALL], W1[2 * ALL:3 * ALL]
    AmB = (A - Bm).astype(np.float32)
    BpC = (Bm + C).astype(np.float32)
    Wx_de, Wzv_de, Wi_de = BpC[0:12], BpC[12:22], BpC[22:26]
    W2d, b2d = dp[1]; W3d, b3d = dp[2]; W4d, b4d = dp[3]
    Wa1, ba1 = ap[0]
    Wa_ai, Wa_x, Wa_z, Wa_v = Wa1[0:26], Wa1[26:38], Wa1[38:44], Wa1[44:48]
    Wa2, ba2 = ap[1]; Wa3, ba3 = ap[2]; Wa4, ba4 = ap[3]
    ba4_eff = (ba4 - one(Wa4)).astype(np.float32)
    b4_eff = (b4d - one(W4d)).astype(np.float32)

    # DE layer-1 "big" lhsT over static rows 0:47 (bias handled in ELU ops)
    lhsT_de1 = np.zeros((47, HID), np.float32)
    lhsT_de1[0:26] = AmB
    lhsT_de1[R_ZVT:R_ZVT + 10] = Wzv_de
    W["de1"] = lhsT_de1
    W["b1row"] = (b1 + Wi_de.T @ ba4_eff).astype(np.float32)
    W["wxde"] = Wx_de
    W["mi"] = (Wa4 @ Wi_de).astype(np.float32)
    W["w2d"] = W2d; W["b2d_eff"] = (b2d - one(W2d)).astype(np.float32)
    W["w3d"] = W3d; W["b3d_eff"] = (b3d - one(W3d)).astype(np.float32)
    W["w4d"] = W4d; W["b4_eff"] = b4_eff

    # AE layer-1 "big" lhsT over static rows 0:37
    lhsT_ae1 = np.zeros((37, HID), np.float32)
    lhsT_ae1[0:26] = Wa_ai
    lhsT_ae1[R_ZV:R_ZV + 6] = Wa_z
    lhsT_ae1[R_ZV + 6:R_ZV + 10] = Wa_v
    W["ae1"] = lhsT_ae1
    W["ba1"] = ba1
    W["wax"] = Wa_x
    W["m4"] = (W4d @ Wa_x).astype(np.float32)
    W["ca2"] = (Wa_x.T @ b4_eff).astype(np.float32)
    W["wa2"] = Wa2; W["ba2_eff"] = (ba2 - one(Wa2)).astype(np.float32)
    W["wa3"] = Wa3; W["ba3_eff"] = (ba3 - one(Wa3)).astype(np.float32)
    W["wa4"] = Wa4; W["ba4_eff"] = ba4_eff

    Wi1, bi1 = ip[0]; Wi2, bi2 = ip[1]; Wi3, bi3 = ip[2]
    W["wi1"] = Wi1; W["bi1"] = bi1
    W["wi2"] = Wi2; W["bi2_eff"] = (bi2 - one(Wi2)).astype(np.float32)
    W["wi3"] = Wi3; W["bi3_eff"] = (bi3 - one(Wi3)).astype(np.float32)

    tg = t[0, :, 0]
    dt_row = np.zeros((T,), np.float32)
    dt_row[0:T - 1] = tg[1:] - tg[:-1]

    stages, stages_bf = [], []
    for c in range(NC_CORES):
        bs = slice(c * BL, (c + 1) * BL)
        S = np.zeros((NROW, T, BL), np.float32)
        zc = z[bs].transpose(2, 1, 0)
        vc = v[bs].transpose(2, 1, 0)
        ic = i[bs].transpose(2, 1, 0)
        S[12:18] = zc[:, 0:1, :]
        S[18:22] = vc[:, 0:1, :]
        S[22:26] = ic[:, 0:1, :]
        S[R_ONE] = 1.0
        S[R_ZV:R_ZV + 6] = zc
        S[R_ZV + 6:R_ZV + 10] = vc
        ev = event_t[bs, 0]
        tk = tg[:-1][:, None]; tk1 = tg[1:][:, None]
        mask = (ev[None, :] > tk) & (ev[None, :] <= tk1)
        zj = z_jump[bs].T[:, None, :]; vj = v_jump[bs].T[:, None, :]
        S[R_ZVT:R_ZVT + 6, :T - 1] = np.where(mask[None], zj, zc[:, :-1])
        S[R_ZVT + 6:R_ZVT + 10, :T - 1] = np.where(mask[None], vj, vc[:, :-1])
        stages.append(np.ascontiguousarray(S.reshape(NROW, NB)))
        Sb = S[0:47].astype(BF16)
        Sb[0:12] = 0
        stages_bf.append(np.ascontiguousarray(Sb.reshape(47, NB)))
    return W, stages, stages_bf, dt_row


# --- weight pack column maps ----------------------------------------------
_CB, _CF = {}, {}


def _build_colmaps():
    cur = 0
    for name, w in [("de1", 128), ("mi", 128), ("w2d", 128), ("w3d", 128),
                    ("ae1", 128), ("m4", 128), ("wa2", 128), ("wa3", 128),
                    ("wa4", 4)]:
        _CB[name] = (cur, w); cur += w
    nb = cur
    cur = 0
    for name, w in [("wxde", 128), ("wax", 128), ("w4d", 12), ("wi1", 128),
                    ("wi2", 128), ("wi3", 12), ("dtpk", T), ("cab", 128),
                    ("cab1", 128), ("b4row", 12), ("ones", 128), ("bc", 16)]:
        _CF[name] = (cur, w); cur += w
    return nb, cur


NWB, NWF = _build_colmaps()

# bias-column indices in "bc": (ts_bias, stt_bias=ts_bias+1) pairs
BC_B1, BC_B1P = 0, 1
BC_B2D, BC_B2DP = 2, 3
BC_B3D, BC_B3DP = 4, 5
BC_BA2, BC_BA2P = 6, 7
BC_BA3, BC_BA3P = 8, 9
BC_BI1, BC_BI1P = 10, 11
BC_BI2, BC_BI2P = 12, 13
BC_BI3 = 14
BC_BA4 = 15


def pack_weights(W, dt_row):
    Pb = np.zeros((128, NWB), BF16)
    Pf = np.zeros((128, NWF), np.float32)

    def putb(name, arr, r0=0):
        c0, w = _CB[name]
        arr = np.asarray(arr, np.float32).astype(BF16)
        Pb[r0:r0 + arr.shape[0], c0:c0 + w] = arr

    def putf(name, arr, r0=0):
        c0, w = _CF[name]
        arr = np.asarray(arr, np.float32)
        Pf[r0:r0 + arr.shape[0], c0:c0 + w] = arr

    putb("de1", W["de1"]); putb("mi", W["mi"]); putb("w2d", W["w2d"])
    putb("w3d", W["w3d"]); putb("ae1", W["ae1"]); putb("m4", W["m4"])
    putb("wa2", W["wa2"]); putb("wa3", W["wa3"]); putb("wa4", W["wa4"])
    putf("wxde", W["wxde"], r0=R_X); putf("wax", W["wax"], r0=R_X)
    putf("w4d", W["w4d"]); putf("wi1", W["wi1"]); putf("wi2", W["wi2"])
    putf("wi3", W["wi3"])
    dtpk = np.zeros((2, T), np.float32)
    dtpk[0] = dt_row; dtpk[1] = 1.0
    putf("dtpk", dtpk)
    putf("cab", np.stack([W["ca2"], W["ba1"]]))
    putf("cab1", np.stack([W["ca2"], W["ba1"] + 1.0]))
    putf("b4row", W["b4_eff"][None, :])
    putf("ones", np.ones((1, 128), np.float32))
    bc = np.zeros((128, 16), np.float32)
    bc[:, BC_B1] = W["b1row"]; bc[:, BC_B1P] = W["b1row"] + 1
    bc[:, BC_B2D] = W["b2d_eff"]; bc[:, BC_B2DP] = W["b2d_eff"] + 1
    bc[:, BC_B3D] = W["b3d_eff"]; bc[:, BC_B3DP] = W["b3d_eff"] + 1
    bc[:, BC_BA2] = W["ba2_eff"]; bc[:, BC_BA2P] = W["ba2_eff"] + 1
    bc[:, BC_BA3] = W["ba3_eff"]; bc[:, BC_BA3P] = W["ba3_eff"] + 1
    bc[:, BC_BI1] = W["bi1"]; bc[:, BC_BI1P] = W["bi1"] + 1
    bc[:, BC_BI2] = W["bi2_eff"]; bc[:, BC_BI2P] = W["bi2_eff"] + 1
    bc[0:12, BC_BI3] = W["bi3_eff"]
    bc[R_I:R_I + 4, BC_BA4] = W["ba4_eff"]
    putf("bc", bc)
    return Pb, Pf


# --- device kernel ---------------------------------------------------------
_NC_CACHE = []


def build_bass():
    nc = bass.Bass()
    stage_d = nc.dram_tensor("stage", [NROW, NB], F32, kind="ExternalInput")
    stagebf_d = nc.dram_tensor("stagebf", [47, NB], BF, kind="ExternalInput")
    wb_d = nc.dram_tensor("wpackb", [128, NWB], BF, kind="ExternalInput")
    wf_d = nc.dram_tensor("wpackf", [128, NWF], F32, kind="ExternalInput")
    out_d = nc.dram_tensor("out", [16, NB], F32, kind="ExternalOutput")

    with TileContext(nc) as tc:
        with (
            tc.tile_pool(name="pers", bufs=1) as pers,
            tc.tile_pool(name="work", bufs=2) as work,
            tc.tile_pool(name="psum", bufs=1, space="PSUM") as psum,
        ):
            wtb = pers.tile([128, NWB], BF, tag="wtb")
            wtf = pers.tile([128, NWF], F32, tag="wtf")
            S = pers.tile([NROW, NB], F32, tag="S")
            Sb = pers.tile([47, NB], BF, tag="Sb")
            nc.gpsimd.dma_start(wtb[:, :], wb_d[:, :])
            nc.gpsimd.dma_start(wtf[:, :], wf_d[:, :])
            nc.gpsimd.dma_start(S[:, :], stage_d[:, :])
            nc.gpsimd.dma_start(Sb[:, :], stagebf_d[:, :])

            def wsb(name, r0, r1):
                c0, w = _CB[name]
                return wtb[r0:r1, c0:c0 + w]

            def wsf(name, r0, r1, c0=0, cn=None):
                cc0, w = _CF[name]
                cn = w if cn is None else cn
                return wtf[r0:r1, cc0 + c0:cc0 + c0 + cn]

            bc0, _ = _CF["bc"]

            def bcol(j, p0=0, p1=128):
                return wtf[p0:p1, bc0 + j:bc0 + j + 1]

            # ---- derived per-step tables (outer products with dt row) ----
            dtb128 = pers.tile([128, T], F32, tag="dtb128")
            dtbx = pers.tile([128, T], F32, tag="dtbx")     # rows 64:76 used
            ae1b = pers.tile([128, T], F32, tag="ae1b")
            ae1b1 = pers.tile([128, T], F32, tag="ae1b1")
            ps = psum.tile([128, T], F32, tag="d1")
            nc.tensor.matmul(ps[:, :], wsf("ones", 0, 1),
                             wsf("dtpk", 0, 1), start=True, stop=True)
            nc.vector.tensor_copy(dtb128[:, :], ps[:, :])
            ps = psum.tile([128, T], F32, tag="d2")
            nc.tensor.matmul(ps[R_X:R_X + 12, :], wsf("b4row", 0, 1),
                             wsf("dtpk", 0, 1), start=True, stop=True,
                             tile_position=(0, R_X))
            nc.vector.tensor_copy(dtbx[R_X:R_X + 12, :], ps[R_X:R_X + 12, :])
            ps = psum.tile([128, T], F32, tag="d3")
            nc.tensor.matmul(ps[:, :], wsf("cab", 0, 2), wsf("dtpk", 0, 2),
                             start=True, stop=True)
            nc.vector.tensor_copy(ae1b[:, :], ps[:, :])
            ps = psum.tile([128, T], F32, tag="a3")
            nc.tensor.matmul(ps[:, :], wsf("cab1", 0, 2), wsf("dtpk", 0, 2),
                             start=True, stop=True)
            nc.vector.tensor_copy(ae1b1[:, :], ps[:, :])

            def blk(r0, r1, k):
                return S[r0:r1, k * BL:(k + 1) * BL]

            def blkb(r0, r1, k):
                return Sb[r0:r1, k * BL:(k + 1) * BL]

            def elu(ps_t, bts, bstt, tag, dt_out=BF):
                """a~ = elu(h)+1 = max(h+1, exp(min(h,0))); h = psum + bias."""
                m = work.tile([128, BL], F32, tag=f"m{tag}")
                E = work.tile([128, BL], F32, tag=f"E{tag}")
                a = work.tile([128, BL], dt_out, tag=f"a{tag}")
                nc.vector.tensor_scalar(m[:, :], ps_t[:, :], bts, 0.0,
                                        Alu.add, Alu.min)
                nc.scalar.activation(E[:, :], m[:, :], Act.Exp)
                nc.vector.scalar_tensor_tensor(a[:, :], ps_t[:, :], bstt,
                                               E[:, :], Alu.add, Alu.max)
                return a

            # ---- preamble: init MLP -> x0 ----
            scr = pers.tile([14, BL], F32, tag="scr")
            nc.gpsimd.dma_start(scr[0:10, :], blk(R_ZV, R_ZV + 10, 0))
            nc.gpsimd.dma_start(scr[10:14, :], blk(22, 26, 0))
            p1 = psum.tile([128, BL], F32, tag="a1")
            nc.tensor.matmul(p1[:, :], wsf("wi1", 0, 14), scr[:, :],
                             start=True, stop=True)
            ai1 = elu(p1, bcol(BC_BI1), bcol(BC_BI1P), "a1", dt_out=F32)
            p2 = psum.tile([128, BL], F32, tag="a2")
            nc.tensor.matmul(p2[:, :], wsf("wi2", 0, 128), ai1[:, :],
                             start=True, stop=True)
            ai2 = elu(p2, bcol(BC_BI2), bcol(BC_BI2P), "a2", dt_out=F32)
            p3 = psum.tile([12, BL], F32, tag="x4")
            nc.tensor.matmul(p3[:, :], wsf("wi3", 0, 128), ai2[:, :],
                             start=True, stop=True)
            x0t = pers.tile([12, BL], F32, tag="x0t")
            x0tb = pers.tile([12, BL], BF, tag="x0tb")
            nc.vector.tensor_scalar(x0t[:, :], p3[:, :], bcol(BC_BI3, 0, 12),
                                    None, Alu.add)
            nc.vector.tensor_copy(x0tb[:, :], x0t[:, :])
            nc.gpsimd.dma_start(blk(R_X, R_X + 12, 0), x0t[:, :])
            nc.gpsimd.dma_start(Sb[0:12, 0:BL], x0tb[:, :])
            # broadcast x0 across all T blocks (doubling SBUF->SBUF DMAs)
            w = BL
            while w < NB:
                nw = min(w, NB - w)
                nc.gpsimd.dma_start(Sb[0:12, w:w + nw], Sb[0:12, 0:nw])
                w += nw

            # ---- preamble: init AE -> i0_hat ----
            pa = psum.tile([128, BL], F32, tag="a1")
            nc.tensor.matmul(pa[:, :], wsb("ae1", 0, 37), blkb(0, 37, 0),
                             start=True, stop=False)
            nc.tensor.matmul(pa[:, :], wsf("wax", R_X, R_X + 12),
                             blk(R_X, R_X + 12, 0), start=False, stop=True)
            aa1 = elu(pa, ae1b[:, T - 1:T], ae1b1[:, T - 1:T], "a1e")
            pa2 = psum.tile([128, BL], F32, tag="a2")
            nc.tensor.matmul(pa2[:, :], wsb("wa2", 0, 128), aa1[:, :],
                             start=True, stop=True)
            aa2 = elu(pa2, bcol(BC_BA2), bcol(BC_BA2P), "a2e")
            pa3 = psum.tile([128, BL], F32, tag="a3")
            nc.tensor.matmul(pa3[:, :], wsb("wa3", 0, 128), aa2[:, :],
                             start=True, stop=True)
            aa3 = elu(pa3, bcol(BC_BA3), bcol(BC_BA3P), "a3e")
            pi4 = psum.tile([128, BL], F32, tag="i4")
            nc.tensor.matmul(pi4[R_I:R_I + 4, :], wsb("wa4", 0, 128),
                             aa3[:, :], start=True, stop=True,
                             tile_position=(0, R_I))
            nc.scalar.activation(blk(R_I, R_I + 4, 0), pi4[R_I:R_I + 4, :],
                                 Act.Identity, bias=bcol(BC_BA4, R_I, R_I + 4))

            # ---- scan ----
            for k in range(T - 1):
                pd1 = psum.tile([128, BL], F32, tag="d1")
                nc.tensor.matmul(pd1[:, :], wsb("de1", 0, 47), blkb(0, 47, k),
                                 start=True, stop=False)
                nc.tensor.matmul(pd1[:, :], wsf("wxde", R_X, R_X + 12),
                                 blk(R_X, R_X + 12, k), start=False,
                                 stop=False)
                nc.tensor.matmul(pd1[:, :], wsb("mi", 0, 128), aa3[:, :],
                                 start=False, stop=True)
                ad1 = elu(pd1, bcol(BC_B1), bcol(BC_B1P), "d1")
                pd2 = psum.tile([128, BL], F32, tag="d2")
                nc.tensor.matmul(pd2[:, :], wsb("w2d", 0, 128), ad1[:, :],
                                 start=True, stop=True)
                ad2 = elu(pd2, bcol(BC_B2D), bcol(BC_B2DP), "d2")
                pd3 = psum.tile([128, BL], F32, tag="d3")
                nc.tensor.matmul(pd3[:, :], wsb("w3d", 0, 128), ad2[:, :],
                                 start=True, stop=True)
                ad3 = elu(pd3, bcol(BC_B3D), bcol(BC_B3DP), "d3", dt_out=F32)
                s3b = work.tile([128, BL], BF, tag="s3b")
                nc.vector.tensor_scalar(s3b[:, :], ad3[:, :],
                                        dtb128[:, k:k + 1], None, Alu.mult)
                s3f = work.tile([128, BL], F32, tag="s3f")
                nc.vector.tensor_scalar(s3f[:, :], ad3[:, :],
                                        dtb128[:, k:k + 1], None, Alu.mult)
                px4 = psum.tile([128, BL], F32, tag="x4")
                nc.tensor.matmul(px4[R_X:R_X + 12, :], wsf("w4d", 0, 128),
                                 s3f[:, :], start=True, stop=True,
                                 tile_position=(0, R_X))
                nc.vector.scalar_tensor_tensor(
                    blk(R_X, R_X + 12, k + 1), px4[R_X:R_X + 12, :],
                    dtbx[R_X:R_X + 12, k:k + 1], blk(R_X, R_X + 12, k),
                    Alu.add, Alu.add)
                pa1 = psum.tile([128, BL], F32, tag="a1")
                nc.tensor.matmul(pa1[:, :], wsb("ae1", 0, 37),
                                 blkb(0, 37, k + 1), start=True, stop=False)
                nc.tensor.matmul(pa1[:, :], wsf("wax", R_X, R_X + 12),
                                 blk(R_X, R_X + 12, k), start=False,
                                 stop=False)
                nc.tensor.matmul(pa1[:, :], wsb("m4", 0, 128), s3b[:, :],
                                 start=False, stop=True)
                aa1 = elu(pa1, ae1b[:, k:k + 1], ae1b1[:, k:k + 1], "a1e")
                pa2 = psum.tile([128, BL], F32, tag="a2")
                nc.tensor.matmul(pa2[:, :], wsb("wa2", 0, 128), aa1[:, :],
                                 start=True, stop=True)
                aa2 = elu(pa2, bcol(BC_BA2), bcol(BC_BA2P), "a2e")
                pa3 = psum.tile([128, BL], F32, tag="a3")
                nc.tensor.matmul(pa3[:, :], wsb("wa3", 0, 128), aa2[:, :],
                                 start=True, stop=True)
                aa3 = elu(pa3, bcol(BC_BA3), bcol(BC_BA3P), "a3e")
                pi4 = psum.tile([128, BL], F32, tag="i4")
                nc.tensor.matmul(pi4[R_I:R_I + 4, :], wsb("wa4", 0, 128),
                                 aa3[:, :], start=True, stop=True,
                                 tile_position=(0, R_I))
                nc.scalar.activation(blk(R_I, R_I + 4, k + 1),
                                     pi4[R_I:R_I + 4, :], Act.Identity,
                                     bias=bcol(BC_BA4, R_I, R_I + 4))

            nc.gpsimd.dma_start(out_d[0:12, :], S[R_X:R_X + 12, :])
            nc.gpsimd.dma_start(out_d[12:16, :], S[R_I:R_I + 4, :])
    return nc


def kernel(**inputs):
    W, stages, stages_bf, dt_row = host_prep(inputs)
    wpb, wpf = pack_weights(W, dt_row)
    if not _NC_CACHE:
        _NC_CACHE.append(build_bass())
    nc = _NC_CACHE[0]
    in_maps = [{"stage": stages[c], "stagebf": stages_bf[c],
                "wpackb": wpb, "wpackf": wpf} for c in range(NC_CORES)]
    res = run_bass_kernel_spmd(nc, in_maps, core_ids=list(range(NC_CORES)))
    xs, is_ = [], []
    for c in range(NC_CORES):
        o = res.results[c]["out"].reshape(16, T, BL)
        xs.append(o[0:12].transpose(2, 1, 0))
        is_.append(o[12:16].transpose(2, 1, 0))
    return (np.ascontiguousarray(np.concatenate(xs, 0)),
            np.ascontiguousarray(np.concatenate(is_, 0)))
